# revision 90
# baseline (speedup 1.0000x reference)
"""Multi-head attention (16 heads, N=2048, D=1024, E=64) on 8 Trainium2 cores.

Head-parallel sharding: core m handles heads (2m, 2m+1), computes its two
heads' attention contexts and a partial o_proj (rows 128m:128m+128 of the
row-sharded o_proj); the host sums the 8 partial bf16 outputs in fp64.

All matmuls run at the full float32r PE rate (1 cycle/row) while keeping
fp32-level accuracy on the precision-critical softmax path:

  x^T arrives as raw fp32 bits typed f32r - the PE's RNE-11 operand
    rounding IS the hi half of an exact split (identical to the host's
    _round11) - plus a precomputed bf16 lo residual.  11+11-bit operands
    multiply exactly, so xh@wh + xl@wh_bf16 + xh@wl is fp32-accurate at
    full PE rate.  wq/wk arrive host-split hi|lo (+ bf16 hi copies for
    the lo-residual term); wv/wo raw (their paths are linear in the
    error so f32r precision suffices).
  projections: qT/kT/vT [E, N] = w^T x^T, d-contraction on PE, both heads
    per matmul (their weight columns are concatenated).  Per-head hi
    copies round PSUM -> f32r on ACT; lo residuals = psum - hi on DVE.
  max-pass: hi-only S'^T[m,q] score tiles (kT_ext_hi @ qT_ext_hi, K=64)
    fold into a per-(chunk, head) DVE running elementwise max (only DVE
    can max against PSUM: Pool has no PSUM access or TensorTensor, ACT
    no max); a Pool partition-max + small DVE negate write -c_q into
    qT_ext row 64 as f32r.  (Error of a few units is fine - softmax
    shift-invariance only needs the shift within ~80 of the true max.)
  scores: S'^T[m,q] = sum_{e<64} k[m,e]q[q,e] - c_q, via e-extension
    (kT_ext row 64 = 1, qT_ext row 64 = -c_q) in two matmuls per tile:
    one stacked K=128 cross-term matmul [kl;kh]@[qh;ql] + one K=65
    kh_ext@qh_ext carrying the max subtraction
  E^T = exp(S'^T / 8) (ScalarE, straight from PSUM)
  ctx^T/Z: lhsT = v_ext [m, 65] (v columns + a ones column) ->
    psum rows 0:63 = ctx^T, row 64 = Z (the softmax denominator),
    accumulated over the 16 m-blocks
  normalize: 1/Z (DVE) broadcast across partitions (Pool) * ctx^T (DVE)
  out_partial[n, :] = ctx_norm_bothheads^T.T @ wo_rows (one K=128 matmul
    per 128-row output block), staged to bf16 SBUF and DMA'd per block
    (bf16 partials halve the output transfers; the host's fp64 sum keeps
    the rounding ~2e-3 of scale, far under the accuracy gate).

Phases are software-pipelined per 512-wide q-chunk: chunk 0's max-pass
rides inside the projection phase (whose n-chunks run in the order
0,1,6,7,2,3,4,5 so the max chain's tail blocks get their k columns
mid-phase); chunk qc+1's max-pass blocks and chunk qc-1's o_proj blocks
ride as fillers interleaved into attention(qc)'s m-block loop, draining
a few iterations before the chunk ends so the next chunk's -c_q row is
staged in time.  The projection head orders the serial DMA transfer
pipe by first use, and the final chunk normalizes head 1 in column
quarters pipelined directly into its o_proj drain.
"""
import sys

sys.path.insert(0, "/opt/trn_rl_repo")

from contextlib import ExitStack

import numpy as np

import concourse.bass as bass
import concourse.mybir as mybir
import concourse.tile as tile
from concourse import bacc
from concourse.bass_utils import run_bass_kernel_spmd
from concourse.masks import make_identity

# problem shapes (hardcoded per contract)
N = 2048
D = 1024
E = 64
H = 16
N_CORES = 8
H_PER_CORE = H // N_CORES  # 2

QC = 512          # q-chunk (moving dim of S'/ctx matmuls)
NQ = N // QC      # 4
MB = 128          # m-block (partition dim of S'^T tiles)
NMB = N // MB     # 16
NPR = NMB // 2    # 8 m-block pairs in the max pass
DCH = D // 128    # 8 d-chunks for projections

F32 = mybir.dt.float32
F32R = mybir.dt.float32r
BF16 = mybir.dt.bfloat16

_CACHE = {}


def build_nc():
    nc = bacc.Bacc(None, target_bir_lowering=False, debug=False)

    # x^T raw fp32 bits typed f32r: the PE's RNE-11 operand rounding IS
    # the hi part of the exact split (identical to the host's _round11);
    # xlb carries the lo residual in bf16 (it is ~2^-11 of x, so bf16
    # keeps the total representation fp32-accurate)
    xt = nc.declare_dram_parameter("xt", [D, N], F32R, isOutput=False)
    xlb = nc.declare_dram_parameter("xlb", [D, N], BF16, isOutput=False)
    wq = nc.declare_dram_parameter("wq", [D, 256], F32R, isOutput=False)
    wk = nc.declare_dram_parameter("wk", [D, 256], F32R, isOutput=False)
    # bf16 hi weights for the xl cross term (bf16 matmul operands must
    # match dtypes; the term is ~2^-11 so bf16 weights are exact enough)
    wqb = nc.declare_dram_parameter("wqb", [D, 128], BF16, isOutput=False)
    wkb = nc.declare_dram_parameter("wkb", [D, 128], BF16, isOutput=False)
    wv = nc.declare_dram_parameter("wv", [D, 128], F32R, isOutput=False)
    wo = nc.declare_dram_parameter("wo", [128, D], F32R, isOutput=False)
    # bf16 partials: the host sums 8 of them in fp64, so the ~2^-9
    # rounding (~2e-3 of the output scale) stays far under the accuracy
    # gate while halving every output DMA transfer
    out = nc.declare_dram_parameter("out", [N, D], BF16, isOutput=True)

    with ExitStack() as ctx:
        tc = ctx.enter_context(tile.TileContext(nc))
        singles = ctx.enter_context(tc.tile_pool(name="singles", bufs=1))
        ps = ctx.enter_context(tc.tile_pool(name="ps", bufs=8, space="PSUM"))
        ex_pool = ctx.enter_context(tc.tile_pool(name="ex", bufs=3))
        bc_pool = ctx.enter_context(tc.tile_pool(name="bc", bufs=1))
        mxs_pool = ctx.enter_context(tc.tile_pool(name="mxs", bufs=2))

        ident = singles.tile([128, 128], F32)

        # long-lived SBUF tensors
        qT_ext = [singles.tile([65, N], F32R, tag=f"qT_ext{h}", name=f"qT_ext{h}")
                  for h in range(2)]
        kT_ext = [singles.tile([65, N], F32R, tag=f"kT_ext{h}", name=f"kT_ext{h}")
                  for h in range(2)]
        # stacked cross-term operands: one K=128 matmul computes
        # kl@qh + kh@ql.  qx = [qh; ql], kx = [kl; kh] (per head).
        qx = [singles.tile([128, N], F32R, tag=f"qx{h}", name=f"qx{h}")
              for h in range(2)]
        kx = [singles.tile([128, N], F32R, tag=f"kx{h}", name=f"kx{h}")
              for h in range(2)]
        v_ext = [singles.tile([128, NMB, 65], F32R, tag=f"v_ext{h}",
                              name=f"v_ext{h}") for h in range(2)]
        ctxn = singles.tile([128, N], F32R, tag="ctxn")
        wo_sb = singles.tile([128, D], F32R, tag="wo_sb")
        # per-(chunk, head) running-max state; lives from a chunk's first
        # max-pass block until its finish
        mp_state = {}

        # ------- max pass: hi-only S'^T tiles -------
        # Only DVE can do elementwise max against a PSUM operand (Pool
        # has neither PSUM access nor TensorTensor, ACT has no max), so
        # each (chunk, head)'s 16 block tiles fold into a DVE running
        # max; one Pool partition-max (SBUF source) + a small DVE negate
        # then write -c_q into qT_ext row 64.
        def mp_block(qc, mb, h):
            qsl = slice(qc * QC, (qc + 1) * QC)
            st = mp_state.setdefault((qc, h), {})
            pt = ps.tile([128, QC], F32, tag="ps", name=f"mp{h}")
            nc.tensor.matmul(
                pt,
                kT_ext[h][0:64, mb * 128:(mb + 1) * 128],
                qT_ext[h][0:64, qsl],
                start=True,
                stop=True,
            )
            if "acc" not in st:
                st["acc"] = mxs_pool.tile(
                    [128, QC], F32, tag=f"acc{h}", name=f"acc{h}",
                    bufs=2)
                nc.vector.tensor_copy(st["acc"], pt)
            else:
                nc.vector.tensor_tensor(
                    out=st["acc"], in0=pt, in1=st["acc"],
                    op=mybir.AluOpType.max)

        def mp_pair(qc, j, h):
            for mb in (2 * j, 2 * j + 1):
                mp_block(qc, mb, h)

        def mp_finish(qc):
            qsl = slice(qc * QC, (qc + 1) * QC)
            for h in range(2):
                st = mp_state.pop((qc, h))
                # partition-max (cross-lane reduce can't negate on hw),
                # then a small negating copy writes the -c_q extension
                # row (f32r cast) on DVE
                cmax = bc_pool.tile([1, QC], F32, tag="cmax", name="cmax")
                nc.gpsimd.tensor_reduce(
                    out=cmax, in_=st["acc"],
                    axis=mybir.AxisListType.C, op=mybir.AluOpType.max,
                )
                nc.vector.tensor_scalar_mul(
                    qT_ext[h][64:65, qsl], cmax, -1.0)

        # ---------------- phase 1: projections ----------------
        with tc.tile_pool(name="ph1", bufs=1) as ph1:
            vT_sb = ph1.tile([128, N], F32, tag="vT_sb")

            def setup_consts():
                # emitted after the first chunk's critical ops so the DVE
                # memsets don't delay the first lo-residual subtract
                make_identity(nc, ident)
                ones_cols = ph1.tile([128, NMB, 1], F32, name="ones_cols")
                nc.vector.memset(ones_cols, 1.0)
                ones_row = ph1.tile([1, N], F32, name="ones_row")
                nc.vector.memset(ones_row, 1.0)
                for h in range(2):
                    # ones row of kT_ext (cast-copy; memset can't write f32r)
                    nc.vector.tensor_copy(kT_ext[h][64:65, :], ones_row)
                    # col 64 of each v_ext block = 1.0
                    nc.vector.tensor_copy(v_ext[h][:, :, 64:65], ones_cols)
                # preload the Exp activation table off the critical path
                dume = ph1.tile([1, 1], F32, name="dume")
                nc.scalar.activation(
                    out=dume, in_=ones_row[:, 0:1],
                    func=mybir.ActivationFunctionType.Exp, scale=0.125,
                )

            w_sb = {}
            for name, w, cols in (("q", wq, 256), ("k", wk, 256), ("v", wv, 128)):
                w_sb[name] = ph1.tile([128, DCH, cols], F32R, tag=f"w_{name}",
                                      name=f"w_{name}")
            for name in ("qb", "kb"):
                w_sb[name] = ph1.tile([128, DCH, 128], BF16, tag=f"w_{name}",
                                      name=f"w_{name}")
            wq_r = wq.rearrange("(c p) e -> p c e", p=128)
            xt_r = xt.rearrange("(c p) n -> p c n", p=128)
            xlb_r = xlb.rearrange("(c p) n -> p c n", p=128)

            # stream x per n-chunk of QCP; the raw fp32 bits typed f32r
            # ARE the hi operand (PE RNE-11 operand rounding = the host's
            # split), the bf16 lo residual arrives precomputed
            QCP = 256
            NQP = N // QCP
            with tc.tile_pool(name="xs", bufs=2) as xs_pool:
                # chunk order: q chunks (0,1) first, then the HIGH k
                # chunks, so the max-pass chain's last blocks get their
                # k columns mid-phase and the chain tail (TT -> C-reduce
                # -> negate) finishes right at phase end
                chunk_order = [0, 1, 6, 7, 2, 3, 4, 5]
                done = set()
                mp_left = set(range(NPR))
                for pos in range(NQP):
                    nchunk = chunk_order[pos]
                    sl = slice(nchunk * QCP, (nchunk + 1) * QCP)
                    xht = xs_pool.tile([128, DCH, QCP], F32R, tag="xht")
                    xlt = xs_pool.tile([128, DCH, QCP], BF16, tag="xlt")
                    def qk_copies(name, pt, deferred=None):
                        dst_ext = qT_ext if name == "q" else kT_ext
                        dst_x = qx if name == "q" else kx
                        hi_rows = (slice(0, 64) if name == "q"
                                   else slice(64, 128))
                        lo_rows = (slice(64, 128) if name == "q"
                                   else slice(0, 64))
                        for h in range(2):
                            hs = slice(h * 64, (h + 1) * 64)
                            # the hi copy rounds PSUM -> f32r on ACT; the
                            # qx hi rows duplicate it SBUF->SBUF on Pool
                            # (no PSUM access there on real hw)
                            nc.scalar.copy(
                                out=dst_ext[h][0:64, sl], in_=pt[hs, :])

                            def lo(h=h, hs=hs):
                                nc.gpsimd.tensor_copy(
                                    dst_x[h][hi_rows, sl],
                                    dst_ext[h][0:64, sl])
                                # lo residual: psum - f32r hi, rounded
                                nc.vector.tensor_sub(
                                    dst_x[h][lo_rows, sl],
                                    pt[hs, :], dst_ext[h][0:64, sl])
                            if deferred is None:
                                lo()
                            else:
                                deferred.append(lo)

                    def v_proj_and_transpose():
                        # v is linear in the error: single f32r term.  The
                        # two m-blocks this chunk covers are transposed
                        # into v_ext right away (spreading the transposes
                        # through phase 1); copies avoid the loaded Pool
                        pt = ps.tile([128, QCP], F32, tag="ps", name="ptv")
                        for c in range(DCH):
                            nc.tensor.matmul(
                                pt, w_sb["v"][:, c, :], xht[:, c, :],
                                start=(c == 0), stop=(c == DCH - 1),
                            )
                        nc.scalar.copy(out=vT_sb[:, sl], in_=pt)
                        for bl in range(2):
                            nb = 2 * nchunk + bl
                            for h in range(2):
                                ptt = ps.tile([128, 64], F32, tag="ps",
                                              name="ptt")
                                nc.tensor.transpose(
                                    ptt,
                                    vT_sb[h * 64:(h + 1) * 64,
                                          nb * 128:(nb + 1) * 128],
                                    ident[h * 64:(h + 1) * 64,
                                          h * 64:(h + 1) * 64],
                                )
                                eng = (nc.scalar.copy if (bl + h) % 2 else
                                       nc.vector.tensor_copy)
                                eng(v_ext[h][:, nb, 0:64], ptt)

                    if nchunk == 0:
                        # DMA order matched to the serial transfer pipe and
                        # the compute order below: minimal bytes before the
                        # first matmul, each stream landing just in time
                        wk_r = wk.rearrange("(c p) e -> p c e", p=128)
                        # wq-hi as ONE issue (the 625ns HWDGE issue cost
                        # exceeds a small slice's transfer, starving the
                        # serial pipe), then the x slices stream
                        nc.sync.dma_start(out=w_sb["q"][:, :, 0:128],
                                          in_=wq_r[:, :, 0:128])
                        for c2 in range(DCH // 2):
                            cs = slice(2 * c2, 2 * c2 + 2)
                            nc.sync.dma_start(out=xht[:, cs, :],
                                              in_=xt_r[:, cs, sl])
                        nc.sync.dma_start(out=w_sb["k"][:, :, 0:128],
                                          in_=wk_r[:, :, 0:128])
                        nc.sync.dma_start(out=w_sb["q"][:, :, 128:256],
                                          in_=wq_r[:, :, 128:256])
                        nc.sync.dma_start(out=w_sb["k"][:, :, 128:256],
                                          in_=wk_r[:, :, 128:256])
                        nc.sync.dma_start(
                            out=w_sb["v"],
                            in_=wv.rearrange("(c p) e -> p c e", p=128))
                        nc.sync.dma_start(out=xlt, in_=xlb_r[:, :, sl])
                        nc.sync.dma_start(
                            out=w_sb["qb"],
                            in_=wqb.rearrange("(c p) e -> p c e", p=128))
                        nc.sync.dma_start(
                            out=w_sb["kb"],
                            in_=wkb.rearrange("(c p) e -> p c e", p=128))
                        # compute in data-arrival order: q-hi, k-hi, the
                        # lo weight terms, then v (needing only wv, it
                        # fills the wait for the bf16 xl stream), and the
                        # xl terms last (accumulation order within a psum
                        # group is free)
                        pts = {"q": ps.tile([128, QCP], F32, tag="ps",
                                            name="ptq"),
                               "k": ps.tile([128, QCP], F32, tag="ps",
                                            name="ptk")}
                        for name in ("q", "k"):
                            for c in range(DCH):
                                nc.tensor.matmul(
                                    pts[name],
                                    w_sb[name][:, c, 0:128],
                                    xht[:, c, :],
                                    start=(c == 0),
                                    stop=False,
                                )
                        for name in ("q", "k"):
                            for c in range(DCH):
                                nc.tensor.matmul(
                                    pts[name],
                                    w_sb[name][:, c, 128:256],
                                    xht[:, c, :],
                                    start=False,
                                    stop=False,
                                )
                            if name == "q":
                                setup_consts()
                        v_proj_and_transpose()
                        for name in ("q", "k"):
                            for c in range(DCH):
                                nc.tensor.matmul(
                                    pts[name],
                                    w_sb[name + "b"][:, c, :],
                                    xlt[:, c, :],
                                    start=False,
                                    stop=(c == DCH - 1),
                                )
                            qk_copies(name, pts[name])
                    else:
                        half = DCH // 2
                        for cs in (slice(0, half), slice(half, DCH)):
                            nc.sync.dma_start(out=xht[:, cs, :],
                                              in_=xt_r[:, cs, sl])
                            nc.sync.dma_start(out=xlt[:, cs, :],
                                              in_=xlb_r[:, cs, sl])
                        if nchunk == 2:
                            nc.sync.dma_start(out=wo_sb, in_=wo[:, :])
                        # on the last chunk the lo-residual copies and the
                        # v work are deferred behind the max-pass tail, so
                        # chunk 0's -c_q row is staged as early as
                        # possible before attention starts (they are only
                        # read late in attention(0))
                        last = pos == NQP - 1
                        deferred = [] if last else None
                        for name in ("q", "k"):
                            pt = ps.tile([128, QCP], F32, tag="ps")
                            i = 0
                            # exact split: xh@wh + xl@wh_bf16 + xh@wl
                            # (weight cols 0:128 = hi both heads,
                            # 128:256 = lo; xh = raw x as f32r)
                            for c in range(DCH):
                                for wop, xt_ in (
                                        (w_sb[name][:, c, 0:128], xht),
                                        (w_sb[name + "b"][:, c, :], xlt),
                                        (w_sb[name][:, c, 128:256], xht)):
                                    nc.tensor.matmul(
                                        pt,
                                        wop,
                                        xt_[:, c, :],
                                        start=(i == 0),
                                        stop=(i == 3 * DCH - 1),
                                    )
                                    i += 1
                            qk_copies(name, pt, deferred)
                        if last:
                            for j in sorted(mp_left):
                                for h in range(2):
                                    mp_pair(0, j, h)
                            mp_left.clear()
                            mp_finish(0)
                            for f in deferred:
                                f()
                        v_proj_and_transpose()
                    # chunk 0 of the max pass rides along with phase 1:
                    # pair j needs exactly k chunk j plus q chunks 0-1
                    done.add(nchunk)
                    if pos >= 1 and pos < NQP - 1 and {0, 1} <= done:
                        for j in sorted(mp_left & done):
                            mp_left.discard(j)
                            for h in range(2):
                                mp_pair(0, j, h)

        def attention_chunk(qc, fillers, seq_heads=False):
            """One q-chunk's attention.  `fillers` is a list of closures
            (next chunk's max-pass pairs, previous chunk's o_proj blocks)
            consumed one per m-block iteration, spreading their PE/DVE/
            Pool load evenly through the chunk."""
            qsl = slice(qc * QC, (qc + 1) * QC)
            ctx_ps = [ps.tile([65, QC], F32, tag="ps", name=f"ctx_ps{h}")
                      for h in range(2)]
            heads_order = ([(mb, h) for mb in range(NMB) for h in range(2)]
                           if not seq_heads else
                           [(mb, h) for h in range(2) for mb in range(NMB)])
            fillers = list(fillers)

            def emit_m1_tail(sp, mb, h):
                # the only matmul that reads row 64 (the -max row); lagging
                # it one m-block behind the cross matmul hides the max
                # staging latency at chunk entry
                nc.tensor.matmul(
                    sp, kT_ext[h][:, mb * 128:(mb + 1) * 128],
                    qT_ext[h][:, qsl],
                    start=False, stop=True,
                )
                et = ex_pool.tile([128, QC], F32R, tag="et", name="et")
                nc.scalar.activation(
                    out=et, in_=sp,
                    func=mybir.ActivationFunctionType.Exp, scale=0.125,
                )
                nc.tensor.matmul(
                    ctx_ps[h], v_ext[h][:, mb, :], et,
                    start=(mb == 0), stop=(mb == NMB - 1),
                )

            lagged = []
            for it, (mb, h) in enumerate(heads_order):
                if it >= 2 and fillers:
                    # drain the filler list ~4 iterations before the chunk
                    # ends so the next chunk's -c_q staging (the last
                    # filler) has settled before its first ext matmul
                    left = max(1, len(heads_order) - 4 - it)
                    n = max(1, -(-len(fillers) // left))
                    for _ in range(n):
                        if fillers:
                            f = fillers.pop(0)
                            if f is not None:
                                f()
                msl = slice(mb * 128, (mb + 1) * 128)
                sp = ps.tile([128, QC], F32, tag="ps", name=f"sp{h}")
                # stacked cross terms first (no row-64 dependency):
                # one K=128 matmul = kl@qh + kh@ql
                nc.tensor.matmul(
                    sp, kx[h][:, msl], qx[h][:, qsl],
                    start=True, stop=False,
                )
                lagged.append((sp, mb, h))
                # lag 2 keeps an extra score tile in flight so the exp
                # latency never backs into the PE; chunk 0 lags deeper
                # still since its -c_q row lands only at the very end of
                # phase 1
                lag = 3
                if len(lagged) > lag:
                    emit_m1_tail(*lagged.pop(0))
                if seq_heads and mb == NMB - 1:
                    while lagged:
                        emit_m1_tail(*lagged.pop(0))
                    if h == 0:
                        # head 1's normalize is emitted by the caller in
                        # column halves, pipelined with its o_proj
                        norm_head(qc, h, ctx_ps)
            while lagged:
                emit_m1_tail(*lagged.pop(0))
            for f in fillers:
                f()
            return ctx_ps

        def norm_head(qc, h, ctx_ps, cl=0, ch=QC):
            q0 = qc * QC
            # normalize: 1/Z broadcast over partitions on Pool
            rz = bc_pool.tile([1, QC], F32, tag="rz")
            nc.vector.reciprocal(out=rz[:, cl:ch], in_=ctx_ps[h][64:65, cl:ch])
            bc_sb = bc_pool.tile([64, QC], F32, tag="bc_sb")
            nc.gpsimd.partition_broadcast(bc_sb[:, cl:ch], rz[:, cl:ch])
            nc.vector.tensor_mul(
                ctxn[h * 64:(h + 1) * 64, q0 + cl:q0 + ch],
                ctx_ps[h][0:64, cl:ch], bc_sb[:, cl:ch]
            )

        _po_eng = [0]

        def oproj_block(qc, nb, dc):
            # one [128, 512] block of this q-chunk's o_proj (both heads
            # fused: K=128), staged through SBUF and DMA'd immediately
            n0 = qc * QC + nb * 128
            po = ps.tile([128, QC], F32, tag="ps", name="po")
            nc.tensor.matmul(
                po,
                ctxn[:, n0:n0 + 128],
                wo_sb[:, dc * QC:(dc + 1) * QC],
                start=True,
                stop=True,
            )
            po_sb = ex_pool.tile([128, QC], BF16, tag="po_sb", bufs=4)
            # staging copies mostly on ACT in chunks that carry max-pass
            # fillers (DVE runs the max chain there); the final chunk has
            # no max-pass, so DVE takes most copies instead
            i = _po_eng[0]
            _po_eng[0] += 1
            dve_share = 3 if qc == NQ - 2 else 0
            if i % 4 < dve_share:
                nc.vector.tensor_copy(po_sb, po)
            else:
                nc.scalar.copy(out=po_sb, in_=po)
            nc.sync.dma_start(
                out=out[n0:n0 + 128, dc * QC:(dc + 1) * QC],
                in_=po_sb)

        def oproj_fillers(qc):
            return [
                (lambda nb=nb, dc=dc: oproj_block(qc, nb, dc))
                for nb in range(QC // 128) for dc in range(D // QC)
            ]

        # pipeline with 1-chunk max-pass lookahead: chunk 0's pairs and
        # finish were hoisted into phase 1; chunk qc+1's pairs + finish
        # ride as early fillers inside attention(qc), chunk qc-1's
        # o_proj blocks as late fillers (their first matmul reads
        # ctxn(qc-1), whose normalize only drains at the start of chunk
        # qc).  The last chunk runs its heads sequentially so head 0's
        # normalize chain overlaps head 1's attention, and its own
        # normalize + o_proj run in column halves to shorten the tail.
        prev = None
        for qc in range(NQ):
            seq = qc == NQ - 1
            mp_f = ([(lambda mb=mb, h=h: mp_block(qc + 1, mb, h))
                     for mb in range(NMB) for h in range(2)]
                    if qc + 1 < NQ else [])
            op_f = oproj_fillers(prev) if prev is not None else []
            # interleave: single-matmul max-pass fillers (each holds just
            # one psum bank until its DVE merge) spread the serial DVE
            # chain; o_proj fillers (ACT-leaning) slot between them, the
            # first held back ~4 iterations until the previous chunk's
            # normalize has drained
            fillers = mp_f[:6] if mp_f else [None] * 6
            rest = mp_f[6:]
            while rest or op_f:
                if op_f:
                    fillers.append(op_f.pop(0))
                for _ in range(3):
                    if rest:
                        fillers.append(rest.pop(0))
            if mp_f:
                fillers.append(lambda: mp_finish(qc + 1))
            ctx_ps = attention_chunk(qc, fillers, seq_heads=seq)
            if not seq:
                for h in range(2):
                    norm_head(qc, h, ctx_ps)
            prev = qc
        # final chunk drain: normalize head 1 in column quarters with the
        # reciprocal / broadcast / multiply stages interleaved (each stage
        # on its engine pipelines across quarters, so the first o_proj
        # block starts after one quarter's chain, not the whole chunk's),
        # then per 128-row block: two matmuls, two staging copies on
        # different engines, ONE [128, 1024] DMA (fewer tail issues)
        q0 = prev * QC
        rz = bc_pool.tile([1, QC], F32, tag="rz")
        bc_sb = bc_pool.tile([64, QC], F32, tag="bc_sb")
        quarters = [slice(i * 128, (i + 1) * 128) for i in range(4)]
        for qs in quarters:
            nc.vector.reciprocal(out=rz[:, qs], in_=ctx_ps[1][64:65, qs])
        for qs in quarters:
            nc.gpsimd.partition_broadcast(bc_sb[:, qs], rz[:, qs])
        for qs in quarters:
            nc.vector.tensor_mul(
                ctxn[64:128, q0 + qs.start:q0 + qs.stop],
                ctx_ps[1][0:64, qs], bc_sb[:, qs])
        for nb in range(QC // 128):
            n0 = prev * QC + nb * 128
            po_nb = ex_pool.tile([128, D], BF16, tag="po_nb", bufs=4,
                                 name="po_nb")
            for dc in range(D // QC):
                po = ps.tile([128, QC], F32, tag="ps", name="po")
                nc.tensor.matmul(
                    po,
                    ctxn[:, n0:n0 + 128],
                    wo_sb[:, dc * QC:(dc + 1) * QC],
                    start=True,
                    stop=True,
                )
                if dc % 2 == 0:
                    nc.vector.tensor_copy(po_nb[:, dc * QC:(dc + 1) * QC], po)
                else:
                    nc.scalar.copy(out=po_nb[:, dc * QC:(dc + 1) * QC],
                                   in_=po)
            dma_eng = (nc.sync, nc.scalar)[nb % 2]
            dma_eng.dma_start(out=out[n0:n0 + 128, :], in_=po_nb)

    nc.compile()
    return nc


def _round11(x):
    # round-to-nearest-even to 11 explicit mantissa bits - exactly the
    # hardware's float32r operand rounding (verified on device)
    u = np.ascontiguousarray(x, dtype=np.float32).view(np.uint32)
    shift = 23 - 11
    add = np.uint32((1 << (shift - 1)) - 1)
    lsb = (u >> np.uint32(shift)) & np.uint32(1)
    mask = np.uint32(~((1 << shift) - 1) & 0xFFFFFFFF)
    return ((u + add + lsb) & mask).view(np.float32)


def _split11(x):
    hi = _round11(x)
    lo = _round11(x.astype(np.float32) - hi)
    return hi, lo


def kernel(x, q_proj, k_proj, v_proj, o_proj):
    import ml_dtypes

    if "nc" not in _CACHE:
        _CACHE["nc"] = build_nc()
    nc = _CACHE["nc"]

    bf16 = ml_dtypes.bfloat16
    xT = np.ascontiguousarray(x.T.astype(np.float32, copy=False))
    # the device uses the raw fp32 bits as the f32r hi operand (PE
    # RNE-11 operand rounding == _round11); ship the lo residual in bf16
    xlbT = np.ascontiguousarray((xT - _round11(xT)).astype(bf16))
    in_maps = []
    for core in range(N_CORES):
        h0 = core * H_PER_CORE

        def wpair(w):
            w2 = np.concatenate([w[h0], w[h0 + 1]], axis=1)  # [D, 128]
            wh, wl = _split11(w2)
            return (np.ascontiguousarray(np.concatenate([wh, wl], axis=1)),
                    np.ascontiguousarray(w2.astype(bf16)))

        wq2, wqb2 = wpair(q_proj)
        wk2, wkb2 = wpair(k_proj)
        m = {
            "xt": xT,
            "xlb": xlbT,
            "wq": wq2,
            "wqb": wqb2,
            "wk": wk2,
            "wkb": wkb2,
            "wv": np.ascontiguousarray(
                np.concatenate([v_proj[h0], v_proj[h0 + 1]], axis=1)),
            "wo": np.ascontiguousarray(o_proj[h0 * 64:(h0 + 2) * 64, :]),
        }
        in_maps.append(m)

    try:
        res = run_bass_kernel_spmd(nc, in_maps, core_ids=list(range(N_CORES)))
    except Exception:
        # one retry: a fresh NRT session recovers transient device faults
        res = run_bass_kernel_spmd(nc, in_maps, core_ids=list(range(N_CORES)))
    _CACHE["last_results"] = res
    acc = np.zeros((N, D), dtype=np.float64)
    for core in range(N_CORES):
        acc += res.results[core]["out"].astype(np.float64)
    return acc.astype(np.float32)


if __name__ == "__main__":
    rng = np.random.default_rng(0)
    ins = {
        "x": rng.standard_normal((N, D), dtype=np.float32),
        "q_proj": rng.standard_normal((H, D, E), dtype=np.float32),
        "k_proj": rng.standard_normal((H, D, E), dtype=np.float32),
        "v_proj": rng.standard_normal((H, D, E), dtype=np.float32),
        "o_proj": rng.standard_normal((D, D), dtype=np.float32),
    }
    out = kernel(**ins)
    print("out", out.shape, out.dtype, np.abs(out).max())


# revision 92
# speedup vs baseline: 1.0022x; 1.0022x over previous
"""Multi-head attention (16 heads, N=2048, D=1024, E=64) on 8 Trainium2 cores.

Head-parallel sharding: core m handles heads (2m, 2m+1), computes its two
heads' attention contexts and a partial o_proj (rows 128m:128m+128 of the
row-sharded o_proj); the host sums the 8 partial bf16 outputs in fp64.

All matmuls run at the full float32r PE rate (1 cycle/row) while keeping
fp32-level accuracy on the precision-critical softmax path:

  x^T arrives as raw fp32 bits typed f32r - the PE's RNE-11 operand
    rounding IS the hi half of an exact split (identical to the host's
    _round11) - plus a precomputed bf16 lo residual.  11+11-bit operands
    multiply exactly, so xh@wh + xl@wh_bf16 + xh@wl is fp32-accurate at
    full PE rate.  wq/wk arrive host-split hi|lo (+ bf16 hi copies for
    the lo-residual term); wv/wo raw (their paths are linear in the
    error so f32r precision suffices).
  projections: qT/kT/vT [E, N] = w^T x^T, d-contraction on PE, both heads
    per matmul (their weight columns are concatenated).  Per-head hi
    copies round PSUM -> f32r on ACT; lo residuals = psum - hi on DVE.
  max-pass: hi-only S'^T[m,q] score tiles (kT_ext_hi @ qT_ext_hi, K=64)
    fold into a per-(chunk, head) DVE running elementwise max (only DVE
    can max against PSUM: Pool has no PSUM access or TensorTensor, ACT
    no max); a Pool partition-max + small DVE negate write -c_q into
    qT_ext row 64 as f32r.  (Error of a few units is fine - softmax
    shift-invariance only needs the shift within ~80 of the true max.)
  scores: S'^T[m,q] = sum_{e<64} k[m,e]q[q,e] - c_q, via e-extension
    (kT_ext row 64 = 1, qT_ext row 64 = -c_q) in two matmuls per tile:
    one stacked K=128 cross-term matmul [kl;kh]@[qh;ql] + one K=65
    kh_ext@qh_ext carrying the max subtraction
  E^T = exp(S'^T / 8) (ScalarE, straight from PSUM)
  ctx^T/Z: lhsT = v_ext [m, 65] (v columns + a ones column) ->
    psum rows 0:63 = ctx^T, row 64 = Z (the softmax denominator),
    accumulated over the 16 m-blocks
  normalize: 1/Z (DVE) broadcast across partitions (Pool) * ctx^T (DVE)
  out_partial[n, :] = ctx_norm_bothheads^T.T @ wo_rows (one K=128 matmul
    per 128-row output block), staged to bf16 SBUF and DMA'd per block
    (bf16 partials halve the output transfers; the host's fp64 sum keeps
    the rounding ~2e-3 of scale, far under the accuracy gate).

Phases are software-pipelined per 512-wide q-chunk: chunk 0's max-pass
rides inside the projection phase (whose n-chunks run in the order
0,1,6,7,2,3,4,5 so the max chain's tail blocks get their k columns
mid-phase); chunk qc+1's max-pass blocks and chunk qc-1's o_proj blocks
ride as fillers interleaved into attention(qc)'s m-block loop, draining
a few iterations before the chunk ends so the next chunk's -c_q row is
staged in time.  The projection head orders the serial DMA transfer
pipe by first use, and the final chunk normalizes head 1 in column
quarters pipelined directly into its o_proj drain.
"""
import sys

sys.path.insert(0, "/opt/trn_rl_repo")

from contextlib import ExitStack

import numpy as np

import concourse.bass as bass
import concourse.mybir as mybir
import concourse.tile as tile
from concourse import bacc
from concourse.bass_utils import run_bass_kernel_spmd
from concourse.masks import make_identity

# problem shapes (hardcoded per contract)
N = 2048
D = 1024
E = 64
H = 16
N_CORES = 8
H_PER_CORE = H // N_CORES  # 2

QC = 512          # q-chunk (moving dim of S'/ctx matmuls)
NQ = N // QC      # 4
MB = 128          # m-block (partition dim of S'^T tiles)
NMB = N // MB     # 16
NPR = NMB // 2    # 8 m-block pairs in the max pass
DCH = D // 128    # 8 d-chunks for projections

F32 = mybir.dt.float32
F32R = mybir.dt.float32r
BF16 = mybir.dt.bfloat16

_CACHE = {}


def build_nc():
    nc = bacc.Bacc(None, target_bir_lowering=False, debug=False)

    # x^T raw fp32 bits typed f32r: the PE's RNE-11 operand rounding IS
    # the hi part of the exact split (identical to the host's _round11);
    # xlb carries the lo residual in bf16 (it is ~2^-11 of x, so bf16
    # keeps the total representation fp32-accurate)
    xt = nc.declare_dram_parameter("xt", [D, N], F32R, isOutput=False)
    xlb = nc.declare_dram_parameter("xlb", [D, N], BF16, isOutput=False)
    wq = nc.declare_dram_parameter("wq", [D, 256], F32R, isOutput=False)
    wk = nc.declare_dram_parameter("wk", [D, 256], F32R, isOutput=False)
    # bf16 hi weights for the xl cross term (bf16 matmul operands must
    # match dtypes; the term is ~2^-11 so bf16 weights are exact enough)
    wqb = nc.declare_dram_parameter("wqb", [D, 128], BF16, isOutput=False)
    wkb = nc.declare_dram_parameter("wkb", [D, 128], BF16, isOutput=False)
    wv = nc.declare_dram_parameter("wv", [D, 128], F32R, isOutput=False)
    wo = nc.declare_dram_parameter("wo", [128, D], F32R, isOutput=False)
    # bf16 partials: the host sums 8 of them in fp64, so the ~2^-9
    # rounding (~2e-3 of the output scale) stays far under the accuracy
    # gate while halving every output DMA transfer
    out = nc.declare_dram_parameter("out", [N, D], BF16, isOutput=True)

    with ExitStack() as ctx:
        tc = ctx.enter_context(tile.TileContext(nc))
        singles = ctx.enter_context(tc.tile_pool(name="singles", bufs=1))
        ps = ctx.enter_context(tc.tile_pool(name="ps", bufs=8, space="PSUM"))
        ex_pool = ctx.enter_context(tc.tile_pool(name="ex", bufs=3))
        bc_pool = ctx.enter_context(tc.tile_pool(name="bc", bufs=1))
        mxs_pool = ctx.enter_context(tc.tile_pool(name="mxs", bufs=2))

        ident = singles.tile([128, 128], F32)

        # long-lived SBUF tensors
        qT_ext = [singles.tile([65, N], F32R, tag=f"qT_ext{h}", name=f"qT_ext{h}")
                  for h in range(2)]
        kT_ext = [singles.tile([65, N], F32R, tag=f"kT_ext{h}", name=f"kT_ext{h}")
                  for h in range(2)]
        # stacked cross-term operands: one K=128 matmul computes
        # kl@qh + kh@ql.  qx = [qh; ql], kx = [kl; kh] (per head).
        qx = [singles.tile([128, N], F32R, tag=f"qx{h}", name=f"qx{h}")
              for h in range(2)]
        kx = [singles.tile([128, N], F32R, tag=f"kx{h}", name=f"kx{h}")
              for h in range(2)]
        v_ext = [singles.tile([128, NMB, 65], F32R, tag=f"v_ext{h}",
                              name=f"v_ext{h}") for h in range(2)]
        ctxn = singles.tile([128, N], F32R, tag="ctxn")
        wo_sb = singles.tile([128, D], F32R, tag="wo_sb")
        # per-(chunk, head) running-max state; lives from a chunk's first
        # max-pass block until its finish
        mp_state = {}

        # ------- max pass: hi-only S'^T tiles -------
        # Only DVE can do elementwise max against a PSUM operand (Pool
        # has neither PSUM access nor TensorTensor, ACT has no max), so
        # each (chunk, head)'s 16 block tiles fold into a DVE running
        # max; one Pool partition-max (SBUF source) + a small DVE negate
        # then write -c_q into qT_ext row 64.
        def mp_block(qc, mb, h):
            qsl = slice(qc * QC, (qc + 1) * QC)
            st = mp_state.setdefault((qc, h), {})
            pt = ps.tile([128, QC], F32, tag="ps", name=f"mp{h}")
            nc.tensor.matmul(
                pt,
                kT_ext[h][0:64, mb * 128:(mb + 1) * 128],
                qT_ext[h][0:64, qsl],
                start=True,
                stop=True,
            )
            if "acc" not in st:
                st["acc"] = mxs_pool.tile(
                    [128, QC], F32, tag=f"acc{h}", name=f"acc{h}",
                    bufs=2)
                nc.vector.tensor_copy(st["acc"], pt)
            else:
                nc.vector.tensor_tensor(
                    out=st["acc"], in0=pt, in1=st["acc"],
                    op=mybir.AluOpType.max)

        def mp_pair(qc, j, h):
            for mb in (2 * j, 2 * j + 1):
                mp_block(qc, mb, h)

        def mp_finish(qc):
            qsl = slice(qc * QC, (qc + 1) * QC)
            for h in range(2):
                st = mp_state.pop((qc, h))
                # partition-max (cross-lane reduce can't negate on hw),
                # then a small negating copy writes the -c_q extension
                # row (f32r cast) on DVE
                cmax = bc_pool.tile([1, QC], F32, tag="cmax", name="cmax")
                nc.gpsimd.tensor_reduce(
                    out=cmax, in_=st["acc"],
                    axis=mybir.AxisListType.C, op=mybir.AluOpType.max,
                )
                nc.vector.tensor_scalar_mul(
                    qT_ext[h][64:65, qsl], cmax, -1.0)

        # ---------------- phase 1: projections ----------------
        with tc.tile_pool(name="ph1", bufs=1) as ph1:
            vT_sb = ph1.tile([128, N], F32, tag="vT_sb")

            def setup_consts():
                # emitted after the first chunk's critical ops so the DVE
                # memsets don't delay the first lo-residual subtract
                make_identity(nc, ident)
                ones_cols = ph1.tile([128, NMB, 1], F32, name="ones_cols")
                nc.vector.memset(ones_cols, 1.0)
                ones_row = ph1.tile([1, N], F32, name="ones_row")
                nc.vector.memset(ones_row, 1.0)
                for h in range(2):
                    # ones row of kT_ext (cast-copy; memset can't write f32r)
                    nc.vector.tensor_copy(kT_ext[h][64:65, :], ones_row)
                    # col 64 of each v_ext block = 1.0
                    nc.vector.tensor_copy(v_ext[h][:, :, 64:65], ones_cols)
                # preload the Exp activation table off the critical path
                dume = ph1.tile([1, 1], F32, name="dume")
                nc.scalar.activation(
                    out=dume, in_=ones_row[:, 0:1],
                    func=mybir.ActivationFunctionType.Exp, scale=0.125,
                )

            w_sb = {}
            for name, w, cols in (("q", wq, 256), ("k", wk, 256), ("v", wv, 128)):
                w_sb[name] = ph1.tile([128, DCH, cols], F32R, tag=f"w_{name}",
                                      name=f"w_{name}")
            for name in ("qb", "kb"):
                w_sb[name] = ph1.tile([128, DCH, 128], BF16, tag=f"w_{name}",
                                      name=f"w_{name}")
            wq_r = wq.rearrange("(c p) e -> p c e", p=128)
            xt_r = xt.rearrange("(c p) n -> p c n", p=128)
            xlb_r = xlb.rearrange("(c p) n -> p c n", p=128)

            # stream x per n-chunk of QCP; the raw fp32 bits typed f32r
            # ARE the hi operand (PE RNE-11 operand rounding = the host's
            # split), the bf16 lo residual arrives precomputed
            QCP = 256
            NQP = N // QCP
            with tc.tile_pool(name="xs", bufs=2) as xs_pool:
                # chunk order: q chunks (0,1) first, then the HIGH k
                # chunks, so the max-pass chain's last blocks get their
                # k columns mid-phase and the chain tail (TT -> C-reduce
                # -> negate) finishes right at phase end
                chunk_order = [0, 1, 6, 7, 2, 3, 4, 5]
                done = set()
                mp_left = set(range(NPR))
                for pos in range(NQP):
                    nchunk = chunk_order[pos]
                    sl = slice(nchunk * QCP, (nchunk + 1) * QCP)
                    xht = xs_pool.tile([128, DCH, QCP], F32R, tag="xht")
                    xlt = xs_pool.tile([128, DCH, QCP], BF16, tag="xlt")
                    def qk_copies(name, pt, deferred=None):
                        dst_ext = qT_ext if name == "q" else kT_ext
                        dst_x = qx if name == "q" else kx
                        hi_rows = (slice(0, 64) if name == "q"
                                   else slice(64, 128))
                        lo_rows = (slice(64, 128) if name == "q"
                                   else slice(0, 64))
                        for h in range(2):
                            hs = slice(h * 64, (h + 1) * 64)
                            # the hi copy rounds PSUM -> f32r on ACT; the
                            # qx hi rows duplicate it SBUF->SBUF on Pool
                            # (no PSUM access there on real hw)
                            nc.scalar.copy(
                                out=dst_ext[h][0:64, sl], in_=pt[hs, :])

                            def lo(h=h, hs=hs):
                                nc.gpsimd.tensor_copy(
                                    dst_x[h][hi_rows, sl],
                                    dst_ext[h][0:64, sl])
                                # lo residual: psum - f32r hi, rounded
                                nc.vector.tensor_sub(
                                    dst_x[h][lo_rows, sl],
                                    pt[hs, :], dst_ext[h][0:64, sl])
                            if deferred is None:
                                lo()
                            else:
                                deferred.append(lo)

                    def v_proj_and_transpose():
                        # v is linear in the error: single f32r term.  The
                        # two m-blocks this chunk covers are transposed
                        # into v_ext right away (spreading the transposes
                        # through phase 1); copies avoid the loaded Pool
                        pt = ps.tile([128, QCP], F32, tag="ps", name="ptv")
                        for c in range(DCH):
                            nc.tensor.matmul(
                                pt, w_sb["v"][:, c, :], xht[:, c, :],
                                start=(c == 0), stop=(c == DCH - 1),
                            )
                        nc.scalar.copy(out=vT_sb[:, sl], in_=pt)
                        for bl in range(2):
                            nb = 2 * nchunk + bl
                            for h in range(2):
                                ptt = ps.tile([128, 64], F32, tag="ps",
                                              name="ptt")
                                nc.tensor.transpose(
                                    ptt,
                                    vT_sb[h * 64:(h + 1) * 64,
                                          nb * 128:(nb + 1) * 128],
                                    ident[h * 64:(h + 1) * 64,
                                          h * 64:(h + 1) * 64],
                                )
                                eng = (nc.scalar.copy if (bl + h) % 2 else
                                       nc.vector.tensor_copy)
                                eng(v_ext[h][:, nb, 0:64], ptt)

                    if nchunk == 0:
                        # DMA order matched to the serial transfer pipe and
                        # the compute order below: minimal bytes before the
                        # first matmul, each stream landing just in time
                        wk_r = wk.rearrange("(c p) e -> p c e", p=128)
                        # wq-hi as ONE issue (the 625ns HWDGE issue cost
                        # exceeds a small slice's transfer, starving the
                        # serial pipe), then the x slices stream
                        nc.sync.dma_start(out=w_sb["q"][:, :, 0:128],
                                          in_=wq_r[:, :, 0:128])
                        for c2 in range(DCH // 2):
                            cs = slice(2 * c2, 2 * c2 + 2)
                            nc.sync.dma_start(out=xht[:, cs, :],
                                              in_=xt_r[:, cs, sl])
                        nc.sync.dma_start(out=w_sb["k"][:, :, 0:128],
                                          in_=wk_r[:, :, 0:128])
                        nc.sync.dma_start(out=w_sb["q"][:, :, 128:256],
                                          in_=wq_r[:, :, 128:256])
                        nc.sync.dma_start(out=w_sb["k"][:, :, 128:256],
                                          in_=wk_r[:, :, 128:256])
                        nc.sync.dma_start(
                            out=w_sb["v"],
                            in_=wv.rearrange("(c p) e -> p c e", p=128))
                        nc.sync.dma_start(out=xlt, in_=xlb_r[:, :, sl])
                        nc.sync.dma_start(
                            out=w_sb["qb"],
                            in_=wqb.rearrange("(c p) e -> p c e", p=128))
                        nc.sync.dma_start(
                            out=w_sb["kb"],
                            in_=wkb.rearrange("(c p) e -> p c e", p=128))
                        # compute in data-arrival order: q-hi, k-hi, the
                        # lo weight terms, then v (needing only wv, it
                        # fills the wait for the bf16 xl stream), and the
                        # xl terms last (accumulation order within a psum
                        # group is free)
                        pts = {"q": ps.tile([128, QCP], F32, tag="ps",
                                            name="ptq"),
                               "k": ps.tile([128, QCP], F32, tag="ps",
                                            name="ptk")}
                        for name in ("q", "k"):
                            for c in range(DCH):
                                nc.tensor.matmul(
                                    pts[name],
                                    w_sb[name][:, c, 0:128],
                                    xht[:, c, :],
                                    start=(c == 0),
                                    stop=False,
                                )
                        for name in ("q", "k"):
                            for c in range(DCH):
                                nc.tensor.matmul(
                                    pts[name],
                                    w_sb[name][:, c, 128:256],
                                    xht[:, c, :],
                                    start=False,
                                    stop=False,
                                )
                            if name == "q":
                                setup_consts()
                        v_proj_and_transpose()
                        for name in ("q", "k"):
                            for c in range(DCH):
                                nc.tensor.matmul(
                                    pts[name],
                                    w_sb[name + "b"][:, c, :],
                                    xlt[:, c, :],
                                    start=False,
                                    stop=(c == DCH - 1),
                                )
                            qk_copies(name, pts[name])
                    else:
                        half = DCH // 2
                        for cs in (slice(0, half), slice(half, DCH)):
                            nc.sync.dma_start(out=xht[:, cs, :],
                                              in_=xt_r[:, cs, sl])
                            nc.sync.dma_start(out=xlt[:, cs, :],
                                              in_=xlb_r[:, cs, sl])
                        if nchunk == 2:
                            nc.sync.dma_start(out=wo_sb, in_=wo[:, :])
                        # on the last chunk the lo-residual copies and the
                        # v work are deferred behind the max-pass tail, so
                        # chunk 0's -c_q row is staged as early as
                        # possible before attention starts (they are only
                        # read late in attention(0))
                        last = pos == NQP - 1
                        deferred = [] if last else None
                        for name in ("q", "k"):
                            pt = ps.tile([128, QCP], F32, tag="ps")
                            i = 0
                            # exact split: xh@wh + xl@wh_bf16 + xh@wl
                            # (weight cols 0:128 = hi both heads,
                            # 128:256 = lo; xh = raw x as f32r)
                            for c in range(DCH):
                                for wop, xt_ in (
                                        (w_sb[name][:, c, 0:128], xht),
                                        (w_sb[name + "b"][:, c, :], xlt),
                                        (w_sb[name][:, c, 128:256], xht)):
                                    nc.tensor.matmul(
                                        pt,
                                        wop,
                                        xt_[:, c, :],
                                        start=(i == 0),
                                        stop=(i == 3 * DCH - 1),
                                    )
                                    i += 1
                            qk_copies(name, pt, deferred)
                        if last:
                            for j in sorted(mp_left):
                                for h in range(2):
                                    mp_pair(0, j, h)
                            mp_left.clear()
                            mp_finish(0)
                            for f in deferred:
                                f()
                        v_proj_and_transpose()
                    # chunk 0 of the max pass rides along with phase 1:
                    # pair j needs exactly k chunk j plus q chunks 0-1
                    done.add(nchunk)
                    if pos >= 1 and pos < NQP - 1 and {0, 1} <= done:
                        for j in sorted(mp_left & done):
                            mp_left.discard(j)
                            for h in range(2):
                                mp_pair(0, j, h)

        def attention_chunk(qc, fillers, seq_heads=False):
            """One q-chunk's attention.  `fillers` is a list of closures
            (next chunk's max-pass pairs, previous chunk's o_proj blocks)
            consumed one per m-block iteration, spreading their PE/DVE/
            Pool load evenly through the chunk."""
            qsl = slice(qc * QC, (qc + 1) * QC)
            ctx_ps = [ps.tile([65, QC], F32, tag="ps", name=f"ctx_ps{h}")
                      for h in range(2)]
            heads_order = ([(mb, h) for mb in range(NMB) for h in range(2)]
                           if not seq_heads else
                           [(mb, h) for h in range(2) for mb in range(NMB)])
            fillers = list(fillers)

            def emit_m1_tail(sp, mb, h):
                # the only matmul that reads row 64 (the -max row); lagging
                # it one m-block behind the cross matmul hides the max
                # staging latency at chunk entry
                nc.tensor.matmul(
                    sp, kT_ext[h][:, mb * 128:(mb + 1) * 128],
                    qT_ext[h][:, qsl],
                    start=False, stop=True,
                )
                et = ex_pool.tile([128, QC], F32R, tag="et", name="et")
                nc.scalar.activation(
                    out=et, in_=sp,
                    func=mybir.ActivationFunctionType.Exp, scale=0.125,
                )
                nc.tensor.matmul(
                    ctx_ps[h], v_ext[h][:, mb, :], et,
                    start=(mb == 0), stop=(mb == NMB - 1),
                )

            lagged = []
            for it, (mb, h) in enumerate(heads_order):
                if it >= 2 and fillers:
                    # drain the filler list ~4 iterations before the chunk
                    # ends so the next chunk's -c_q staging (the last
                    # filler) has settled before its first ext matmul
                    left = max(1, len(heads_order) - 4 - it)
                    n = max(1, -(-len(fillers) // left))
                    for _ in range(n):
                        if fillers:
                            f = fillers.pop(0)
                            if f is not None:
                                f()
                msl = slice(mb * 128, (mb + 1) * 128)
                sp = ps.tile([128, QC], F32, tag="ps", name=f"sp{h}")
                # stacked cross terms first (no row-64 dependency):
                # one K=128 matmul = kl@qh + kh@ql
                nc.tensor.matmul(
                    sp, kx[h][:, msl], qx[h][:, qsl],
                    start=True, stop=False,
                )
                lagged.append((sp, mb, h))
                # lag 2 keeps an extra score tile in flight so the exp
                # latency never backs into the PE; chunk 0 lags deeper
                # still since its -c_q row lands only at the very end of
                # phase 1
                lag = 3
                if len(lagged) > lag:
                    emit_m1_tail(*lagged.pop(0))
                if seq_heads and mb == NMB - 1:
                    while lagged:
                        emit_m1_tail(*lagged.pop(0))
                    if h == 0:
                        # head 1's normalize is emitted by the caller in
                        # column halves, pipelined with its o_proj
                        norm_head(qc, h, ctx_ps)
            while lagged:
                emit_m1_tail(*lagged.pop(0))
            for f in fillers:
                f()
            return ctx_ps

        def norm_head(qc, h, ctx_ps, cl=0, ch=QC):
            q0 = qc * QC
            # normalize: 1/Z broadcast over partitions on Pool
            rz = bc_pool.tile([1, QC], F32, tag="rz")
            nc.vector.reciprocal(out=rz[:, cl:ch], in_=ctx_ps[h][64:65, cl:ch])
            bc_sb = bc_pool.tile([64, QC], F32, tag="bc_sb")
            nc.gpsimd.partition_broadcast(bc_sb[:, cl:ch], rz[:, cl:ch])
            nc.vector.tensor_mul(
                ctxn[h * 64:(h + 1) * 64, q0 + cl:q0 + ch],
                ctx_ps[h][0:64, cl:ch], bc_sb[:, cl:ch]
            )

        _po_eng = [0]

        def oproj_block(qc, nb, dc):
            # one [128, 512] block of this q-chunk's o_proj (both heads
            # fused: K=128), staged through SBUF and DMA'd immediately
            n0 = qc * QC + nb * 128
            po = ps.tile([128, QC], F32, tag="ps", name="po")
            nc.tensor.matmul(
                po,
                ctxn[:, n0:n0 + 128],
                wo_sb[:, dc * QC:(dc + 1) * QC],
                start=True,
                stop=True,
            )
            po_sb = ex_pool.tile([128, QC], BF16, tag="po_sb", bufs=4)
            # staging copies mostly on ACT in chunks that carry max-pass
            # fillers (DVE runs the max chain there); the final chunk has
            # no max-pass, so DVE takes most copies instead
            i = _po_eng[0]
            _po_eng[0] += 1
            dve_share = 3 if qc == NQ - 2 else 0
            if i % 4 < dve_share:
                nc.vector.tensor_copy(po_sb, po)
            else:
                nc.scalar.copy(out=po_sb, in_=po)
            nc.sync.dma_start(
                out=out[n0:n0 + 128, dc * QC:(dc + 1) * QC],
                in_=po_sb)

        def oproj_fillers(qc):
            return [
                (lambda nb=nb, dc=dc: oproj_block(qc, nb, dc))
                for nb in range(QC // 128) for dc in range(D // QC)
            ]

        # pipeline with 1-chunk max-pass lookahead: chunk 0's pairs and
        # finish were hoisted into phase 1; chunk qc+1's pairs + finish
        # ride as early fillers inside attention(qc), chunk qc-1's
        # o_proj blocks as late fillers (their first matmul reads
        # ctxn(qc-1), whose normalize only drains at the start of chunk
        # qc).  The last chunk runs its heads sequentially so head 0's
        # normalize chain overlaps head 1's attention, and its own
        # normalize + o_proj run in column halves to shorten the tail.
        prev = None
        for qc in range(NQ):
            seq = qc == NQ - 1
            mp_f = ([(lambda mb=mb, h=h: mp_block(qc + 1, mb, h))
                     for mb in range(NMB) for h in range(2)]
                    if qc + 1 < NQ else [])
            op_f = oproj_fillers(prev) if prev is not None else []
            # interleave: single-matmul max-pass fillers (each holds just
            # one psum bank until its DVE merge) spread the serial DVE
            # chain; o_proj fillers (ACT-leaning) slot between them, the
            # first held back ~4 iterations until the previous chunk's
            # normalize has drained
            fillers = mp_f[:8] if mp_f else [None] * 8
            rest = mp_f[8:]
            while rest or op_f:
                if op_f:
                    fillers.append(op_f.pop(0))
                for _ in range(2):
                    if rest:
                        fillers.append(rest.pop(0))
            if mp_f:
                fillers.append(lambda: mp_finish(qc + 1))
            ctx_ps = attention_chunk(qc, fillers, seq_heads=seq)
            if not seq:
                for h in range(2):
                    norm_head(qc, h, ctx_ps)
            prev = qc
        # final chunk drain: normalize head 1 in column quarters with the
        # reciprocal / broadcast / multiply stages interleaved (each stage
        # on its engine pipelines across quarters, so the first o_proj
        # block starts after one quarter's chain, not the whole chunk's),
        # then per 128-row block: two matmuls, two staging copies on
        # different engines, ONE [128, 1024] DMA (fewer tail issues)
        q0 = prev * QC
        rz = bc_pool.tile([1, QC], F32, tag="rz")
        bc_sb = bc_pool.tile([64, QC], F32, tag="bc_sb")
        quarters = [slice(i * 128, (i + 1) * 128) for i in range(4)]
        for qs in quarters:
            nc.vector.reciprocal(out=rz[:, qs], in_=ctx_ps[1][64:65, qs])
        for qs in quarters:
            nc.gpsimd.partition_broadcast(bc_sb[:, qs], rz[:, qs])
        for qs in quarters:
            nc.vector.tensor_mul(
                ctxn[64:128, q0 + qs.start:q0 + qs.stop],
                ctx_ps[1][0:64, qs], bc_sb[:, qs])
        for nb in range(QC // 128):
            n0 = prev * QC + nb * 128
            po_nb = ex_pool.tile([128, D], BF16, tag="po_nb", bufs=4,
                                 name="po_nb")
            for dc in range(D // QC):
                po = ps.tile([128, QC], F32, tag="ps", name="po")
                nc.tensor.matmul(
                    po,
                    ctxn[:, n0:n0 + 128],
                    wo_sb[:, dc * QC:(dc + 1) * QC],
                    start=True,
                    stop=True,
                )
                if dc % 2 == 0:
                    nc.vector.tensor_copy(po_nb[:, dc * QC:(dc + 1) * QC], po)
                else:
                    nc.scalar.copy(out=po_nb[:, dc * QC:(dc + 1) * QC],
                                   in_=po)
            dma_eng = (nc.sync, nc.scalar)[nb % 2]
            dma_eng.dma_start(out=out[n0:n0 + 128, :], in_=po_nb)

    nc.compile()
    return nc


def _round11(x):
    # round-to-nearest-even to 11 explicit mantissa bits - exactly the
    # hardware's float32r operand rounding (verified on device)
    u = np.ascontiguousarray(x, dtype=np.float32).view(np.uint32)
    shift = 23 - 11
    add = np.uint32((1 << (shift - 1)) - 1)
    lsb = (u >> np.uint32(shift)) & np.uint32(1)
    mask = np.uint32(~((1 << shift) - 1) & 0xFFFFFFFF)
    return ((u + add + lsb) & mask).view(np.float32)


def _split11(x):
    hi = _round11(x)
    lo = _round11(x.astype(np.float32) - hi)
    return hi, lo


def kernel(x, q_proj, k_proj, v_proj, o_proj):
    import ml_dtypes

    if "nc" not in _CACHE:
        _CACHE["nc"] = build_nc()
    nc = _CACHE["nc"]

    bf16 = ml_dtypes.bfloat16
    xT = np.ascontiguousarray(x.T.astype(np.float32, copy=False))
    # the device uses the raw fp32 bits as the f32r hi operand (PE
    # RNE-11 operand rounding == _round11); ship the lo residual in bf16
    xlbT = np.ascontiguousarray((xT - _round11(xT)).astype(bf16))
    in_maps = []
    for core in range(N_CORES):
        h0 = core * H_PER_CORE

        def wpair(w):
            w2 = np.concatenate([w[h0], w[h0 + 1]], axis=1)  # [D, 128]
            wh, wl = _split11(w2)
            return (np.ascontiguousarray(np.concatenate([wh, wl], axis=1)),
                    np.ascontiguousarray(w2.astype(bf16)))

        wq2, wqb2 = wpair(q_proj)
        wk2, wkb2 = wpair(k_proj)
        m = {
            "xt": xT,
            "xlb": xlbT,
            "wq": wq2,
            "wqb": wqb2,
            "wk": wk2,
            "wkb": wkb2,
            "wv": np.ascontiguousarray(
                np.concatenate([v_proj[h0], v_proj[h0 + 1]], axis=1)),
            "wo": np.ascontiguousarray(o_proj[h0 * 64:(h0 + 2) * 64, :]),
        }
        in_maps.append(m)

    try:
        res = run_bass_kernel_spmd(nc, in_maps, core_ids=list(range(N_CORES)))
    except Exception:
        # one retry: a fresh NRT session recovers transient device faults
        res = run_bass_kernel_spmd(nc, in_maps, core_ids=list(range(N_CORES)))
    _CACHE["last_results"] = res
    acc = np.zeros((N, D), dtype=np.float64)
    for core in range(N_CORES):
        acc += res.results[core]["out"].astype(np.float64)
    return acc.astype(np.float32)


if __name__ == "__main__":
    rng = np.random.default_rng(0)
    ins = {
        "x": rng.standard_normal((N, D), dtype=np.float32),
        "q_proj": rng.standard_normal((H, D, E), dtype=np.float32),
        "k_proj": rng.standard_normal((H, D, E), dtype=np.float32),
        "v_proj": rng.standard_normal((H, D, E), dtype=np.float32),
        "o_proj": rng.standard_normal((D, D), dtype=np.float32),
    }
    out = kernel(**ins)
    print("out", out.shape, out.dtype, np.abs(out).max())


# revision 94
# speedup vs baseline: 1.0048x; 1.0026x over previous
"""Multi-head attention (16 heads, N=2048, D=1024, E=64) on 8 Trainium2 cores.

Head-parallel sharding: core m handles heads (2m, 2m+1), computes its two
heads' attention contexts and a partial o_proj (rows 128m:128m+128 of the
row-sharded o_proj); the host sums the 8 partial bf16 outputs in fp64.

All matmuls run at the full float32r PE rate (1 cycle/row) while keeping
fp32-level accuracy on the precision-critical softmax path:

  x^T arrives as raw fp32 bits typed f32r - the PE's RNE-11 operand
    rounding IS the hi half of an exact split (identical to the host's
    _round11) - plus a precomputed bf16 lo residual.  11+11-bit operands
    multiply exactly, so xh@wh + xl@wh_bf16 + xh@wl is fp32-accurate at
    full PE rate.  wq/wk arrive host-split hi|lo (+ bf16 hi copies for
    the lo-residual term); wv/wo raw (their paths are linear in the
    error so f32r precision suffices).
  projections: qT/kT/vT [E, N] = w^T x^T, d-contraction on PE, both heads
    per matmul (their weight columns are concatenated).  Per-head hi
    copies round PSUM -> f32r on ACT; lo residuals = psum - hi on DVE.
  max-pass: hi-only S'^T[m,q] score tiles (kT_ext_hi @ qT_ext_hi, K=64)
    fold into a per-(chunk, head) DVE running elementwise max (only DVE
    can max against PSUM: Pool has no PSUM access or TensorTensor, ACT
    no max); a Pool partition-max + small DVE negate write -c_q into
    qT_ext row 64 as f32r.  (Error of a few units is fine - softmax
    shift-invariance only needs the shift within ~80 of the true max.)
  scores: S'^T[m,q] = sum_{e<64} k[m,e]q[q,e] - c_q, via e-extension
    (kT_ext row 64 = 1, qT_ext row 64 = -c_q) in two matmuls per tile:
    one stacked K=128 cross-term matmul [kl;kh]@[qh;ql] + one K=65
    kh_ext@qh_ext carrying the max subtraction
  E^T = exp(S'^T / 8) (ScalarE, straight from PSUM)
  ctx^T/Z: lhsT = v_ext [m, 65] (v columns + a ones column) ->
    psum rows 0:63 = ctx^T, row 64 = Z (the softmax denominator),
    accumulated over the 16 m-blocks
  normalize: 1/Z (DVE) broadcast across partitions (Pool) * ctx^T (DVE)
  out_partial[n, :] = ctx_norm_bothheads^T.T @ wo_rows (one K=128 matmul
    per 128-row output block), staged to bf16 SBUF and DMA'd per block
    (bf16 partials halve the output transfers; the host's fp64 sum keeps
    the rounding ~2e-3 of scale, far under the accuracy gate).

Phases are software-pipelined per 512-wide q-chunk: chunk 0's max-pass
rides inside the projection phase (whose n-chunks run in the order
0,1,6,7,2,3,4,5 so the max chain's tail blocks get their k columns
mid-phase); chunk qc+1's max-pass blocks and chunk qc-1's o_proj blocks
ride as fillers interleaved into attention(qc)'s m-block loop, draining
a few iterations before the chunk ends so the next chunk's -c_q row is
staged in time.  The projection head orders the serial DMA transfer
pipe by first use, and the final chunk normalizes head 1 in column
quarters pipelined directly into its o_proj drain.
"""
import sys

sys.path.insert(0, "/opt/trn_rl_repo")

from contextlib import ExitStack

import numpy as np

import concourse.bass as bass
import concourse.mybir as mybir
import concourse.tile as tile
from concourse import bacc
from concourse.bass_utils import run_bass_kernel_spmd
from concourse.masks import make_identity

# problem shapes (hardcoded per contract)
N = 2048
D = 1024
E = 64
H = 16
N_CORES = 8
H_PER_CORE = H // N_CORES  # 2

QC = 512          # q-chunk (moving dim of S'/ctx matmuls)
NQ = N // QC      # 4
MB = 128          # m-block (partition dim of S'^T tiles)
NMB = N // MB     # 16
NPR = NMB // 2    # 8 m-block pairs in the max pass
DCH = D // 128    # 8 d-chunks for projections

F32 = mybir.dt.float32
F32R = mybir.dt.float32r
BF16 = mybir.dt.bfloat16

_CACHE = {}


def build_nc():
    nc = bacc.Bacc(None, target_bir_lowering=False, debug=False)

    # x^T raw fp32 bits typed f32r: the PE's RNE-11 operand rounding IS
    # the hi part of the exact split (identical to the host's _round11);
    # xlb carries the lo residual in bf16 (it is ~2^-11 of x, so bf16
    # keeps the total representation fp32-accurate)
    xt = nc.declare_dram_parameter("xt", [D, N], F32R, isOutput=False)
    xlb = nc.declare_dram_parameter("xlb", [D, N], BF16, isOutput=False)
    wq = nc.declare_dram_parameter("wq", [D, 256], F32R, isOutput=False)
    wk = nc.declare_dram_parameter("wk", [D, 256], F32R, isOutput=False)
    # bf16 hi weights for the xl cross term (bf16 matmul operands must
    # match dtypes; the term is ~2^-11 so bf16 weights are exact enough)
    wqb = nc.declare_dram_parameter("wqb", [D, 128], BF16, isOutput=False)
    wkb = nc.declare_dram_parameter("wkb", [D, 128], BF16, isOutput=False)
    wv = nc.declare_dram_parameter("wv", [D, 128], F32R, isOutput=False)
    wo = nc.declare_dram_parameter("wo", [128, D], F32R, isOutput=False)
    # bf16 partials: the host sums 8 of them in fp64, so the ~2^-9
    # rounding (~2e-3 of the output scale) stays far under the accuracy
    # gate while halving every output DMA transfer
    out = nc.declare_dram_parameter("out", [N, D], BF16, isOutput=True)

    with ExitStack() as ctx:
        tc = ctx.enter_context(tile.TileContext(nc))
        singles = ctx.enter_context(tc.tile_pool(name="singles", bufs=1))
        ps = ctx.enter_context(tc.tile_pool(name="ps", bufs=8, space="PSUM"))
        ex_pool = ctx.enter_context(tc.tile_pool(name="ex", bufs=3))
        bc_pool = ctx.enter_context(tc.tile_pool(name="bc", bufs=1))
        mxs_pool = ctx.enter_context(tc.tile_pool(name="mxs", bufs=2))

        ident = singles.tile([128, 128], F32)

        # long-lived SBUF tensors
        qT_ext = [singles.tile([65, N], F32R, tag=f"qT_ext{h}", name=f"qT_ext{h}")
                  for h in range(2)]
        kT_ext = [singles.tile([65, N], F32R, tag=f"kT_ext{h}", name=f"kT_ext{h}")
                  for h in range(2)]
        # stacked cross-term operands: one K=128 matmul computes
        # kl@qh + kh@ql.  qx = [qh; ql], kx = [kl; kh] (per head).
        qx = [singles.tile([128, N], F32R, tag=f"qx{h}", name=f"qx{h}")
              for h in range(2)]
        kx = [singles.tile([128, N], F32R, tag=f"kx{h}", name=f"kx{h}")
              for h in range(2)]
        v_ext = [singles.tile([128, NMB, 65], F32R, tag=f"v_ext{h}",
                              name=f"v_ext{h}") for h in range(2)]
        ctxn = singles.tile([128, N], F32R, tag="ctxn")
        wo_sb = singles.tile([128, D], F32R, tag="wo_sb")
        # per-(chunk, head) running-max state; lives from a chunk's first
        # max-pass block until its finish
        mp_state = {}

        # ------- max pass: hi-only S'^T tiles -------
        # Only DVE can do elementwise max against a PSUM operand (Pool
        # has neither PSUM access nor TensorTensor, ACT has no max), so
        # each (chunk, head)'s 16 block tiles fold into a DVE running
        # max; one Pool partition-max (SBUF source) + a small DVE negate
        # then write -c_q into qT_ext row 64.
        def mp_block(qc, mb, h):
            qsl = slice(qc * QC, (qc + 1) * QC)
            st = mp_state.setdefault((qc, h), {})
            pt = ps.tile([128, QC], F32, tag="ps", name=f"mp{h}")
            nc.tensor.matmul(
                pt,
                kT_ext[h][0:64, mb * 128:(mb + 1) * 128],
                qT_ext[h][0:64, qsl],
                start=True,
                stop=True,
            )
            if "acc" not in st:
                st["acc"] = mxs_pool.tile(
                    [128, QC], F32, tag=f"acc{h}", name=f"acc{h}",
                    bufs=2)
                nc.vector.tensor_copy(st["acc"], pt)
            else:
                nc.vector.tensor_tensor(
                    out=st["acc"], in0=pt, in1=st["acc"],
                    op=mybir.AluOpType.max)

        def mp_pair(qc, j, h):
            for mb in (2 * j, 2 * j + 1):
                mp_block(qc, mb, h)

        def mp_finish(qc):
            qsl = slice(qc * QC, (qc + 1) * QC)
            for h in range(2):
                st = mp_state.pop((qc, h))
                # partition-max (cross-lane reduce can't negate on hw),
                # then a small negating copy writes the -c_q extension
                # row (f32r cast) on DVE
                cmax = bc_pool.tile([1, QC], F32, tag="cmax", name="cmax")
                nc.gpsimd.tensor_reduce(
                    out=cmax, in_=st["acc"],
                    axis=mybir.AxisListType.C, op=mybir.AluOpType.max,
                )
                nc.vector.tensor_scalar_mul(
                    qT_ext[h][64:65, qsl], cmax, -1.0)

        # ---------------- phase 1: projections ----------------
        with tc.tile_pool(name="ph1", bufs=1) as ph1:
            vT_sb = ph1.tile([128, N], F32, tag="vT_sb")

            def setup_consts():
                # emitted after the first chunk's critical ops so the DVE
                # memsets don't delay the first lo-residual subtract
                make_identity(nc, ident)
                ones_cols = ph1.tile([128, NMB, 1], F32, name="ones_cols")
                nc.vector.memset(ones_cols, 1.0)
                ones_row = ph1.tile([1, N], F32, name="ones_row")
                nc.vector.memset(ones_row, 1.0)
                for h in range(2):
                    # ones row of kT_ext (cast-copy; memset can't write f32r)
                    nc.vector.tensor_copy(kT_ext[h][64:65, :], ones_row)
                    # col 64 of each v_ext block = 1.0
                    nc.vector.tensor_copy(v_ext[h][:, :, 64:65], ones_cols)
                # preload the Exp activation table off the critical path
                dume = ph1.tile([1, 1], F32, name="dume")
                nc.scalar.activation(
                    out=dume, in_=ones_row[:, 0:1],
                    func=mybir.ActivationFunctionType.Exp, scale=0.125,
                )

            w_sb = {}
            for name, w, cols in (("q", wq, 256), ("k", wk, 256), ("v", wv, 128)):
                w_sb[name] = ph1.tile([128, DCH, cols], F32R, tag=f"w_{name}",
                                      name=f"w_{name}")
            for name in ("qb", "kb"):
                w_sb[name] = ph1.tile([128, DCH, 128], BF16, tag=f"w_{name}",
                                      name=f"w_{name}")
            wq_r = wq.rearrange("(c p) e -> p c e", p=128)
            xt_r = xt.rearrange("(c p) n -> p c n", p=128)
            xlb_r = xlb.rearrange("(c p) n -> p c n", p=128)

            # stream x per n-chunk of QCP; the raw fp32 bits typed f32r
            # ARE the hi operand (PE RNE-11 operand rounding = the host's
            # split), the bf16 lo residual arrives precomputed
            QCP = 256
            NQP = N // QCP
            with tc.tile_pool(name="xs", bufs=2) as xs_pool:
                # chunk order: q chunks (0,1) first, then the HIGH k
                # chunks, so the max-pass chain's last blocks get their
                # k columns mid-phase and the chain tail (TT -> C-reduce
                # -> negate) finishes right at phase end
                chunk_order = [0, 1, 6, 7, 2, 3, 4, 5]
                done = set()
                mp_left = set(range(NPR))
                for pos in range(NQP):
                    nchunk = chunk_order[pos]
                    sl = slice(nchunk * QCP, (nchunk + 1) * QCP)
                    xht = xs_pool.tile([128, DCH, QCP], F32R, tag="xht")
                    xlt = xs_pool.tile([128, DCH, QCP], BF16, tag="xlt")
                    def qk_copies(name, pt, deferred=None):
                        dst_ext = qT_ext if name == "q" else kT_ext
                        dst_x = qx if name == "q" else kx
                        hi_rows = (slice(0, 64) if name == "q"
                                   else slice(64, 128))
                        lo_rows = (slice(64, 128) if name == "q"
                                   else slice(0, 64))
                        for h in range(2):
                            hs = slice(h * 64, (h + 1) * 64)
                            # the hi copy rounds PSUM -> f32r on ACT; the
                            # qx hi rows duplicate it SBUF->SBUF on Pool
                            # (no PSUM access there on real hw)
                            nc.scalar.copy(
                                out=dst_ext[h][0:64, sl], in_=pt[hs, :])

                            def lo(h=h, hs=hs):
                                nc.gpsimd.tensor_copy(
                                    dst_x[h][hi_rows, sl],
                                    dst_ext[h][0:64, sl])
                                # lo residual: psum - f32r hi, rounded
                                nc.vector.tensor_sub(
                                    dst_x[h][lo_rows, sl],
                                    pt[hs, :], dst_ext[h][0:64, sl])
                            if deferred is None:
                                lo()
                            else:
                                deferred.append(lo)

                    def v_proj_and_transpose():
                        # v is linear in the error: single f32r term.  The
                        # two m-blocks this chunk covers are transposed
                        # into v_ext right away (spreading the transposes
                        # through phase 1); copies avoid the loaded Pool
                        pt = ps.tile([128, QCP], F32, tag="ps", name="ptv")
                        for c in range(DCH):
                            nc.tensor.matmul(
                                pt, w_sb["v"][:, c, :], xht[:, c, :],
                                start=(c == 0), stop=(c == DCH - 1),
                            )
                        nc.scalar.copy(out=vT_sb[:, sl], in_=pt)
                        for bl in range(2):
                            nb = 2 * nchunk + bl
                            for h in range(2):
                                ptt = ps.tile([128, 64], F32, tag="ps",
                                              name="ptt")
                                nc.tensor.transpose(
                                    ptt,
                                    vT_sb[h * 64:(h + 1) * 64,
                                          nb * 128:(nb + 1) * 128],
                                    ident[h * 64:(h + 1) * 64,
                                          h * 64:(h + 1) * 64],
                                )
                                eng = (nc.scalar.copy if (bl + h) % 2 else
                                       nc.vector.tensor_copy)
                                eng(v_ext[h][:, nb, 0:64], ptt)

                    if nchunk == 0:
                        # DMA order matched to the serial transfer pipe and
                        # the compute order below: minimal bytes before the
                        # first matmul, each stream landing just in time
                        wk_r = wk.rearrange("(c p) e -> p c e", p=128)
                        # wq-hi as ONE issue (the 625ns HWDGE issue cost
                        # exceeds a small slice's transfer, starving the
                        # serial pipe), then the x slices stream
                        nc.sync.dma_start(out=w_sb["q"][:, :, 0:128],
                                          in_=wq_r[:, :, 0:128])
                        for c2 in range(DCH // 2):
                            cs = slice(2 * c2, 2 * c2 + 2)
                            nc.sync.dma_start(out=xht[:, cs, :],
                                              in_=xt_r[:, cs, sl])
                        nc.sync.dma_start(out=w_sb["k"][:, :, 0:128],
                                          in_=wk_r[:, :, 0:128])
                        nc.sync.dma_start(out=w_sb["q"][:, :, 128:256],
                                          in_=wq_r[:, :, 128:256])
                        nc.sync.dma_start(out=w_sb["k"][:, :, 128:256],
                                          in_=wk_r[:, :, 128:256])
                        nc.sync.dma_start(
                            out=w_sb["v"],
                            in_=wv.rearrange("(c p) e -> p c e", p=128))
                        nc.sync.dma_start(out=xlt, in_=xlb_r[:, :, sl])
                        nc.sync.dma_start(
                            out=w_sb["qb"],
                            in_=wqb.rearrange("(c p) e -> p c e", p=128))
                        nc.sync.dma_start(
                            out=w_sb["kb"],
                            in_=wkb.rearrange("(c p) e -> p c e", p=128))
                        # compute in data-arrival order: q-hi, k-hi, the
                        # lo weight terms, then v (needing only wv, it
                        # fills the wait for the bf16 xl stream), and the
                        # xl terms last (accumulation order within a psum
                        # group is free)
                        pts = {"q": ps.tile([128, QCP], F32, tag="ps",
                                            name="ptq"),
                               "k": ps.tile([128, QCP], F32, tag="ps",
                                            name="ptk")}
                        for name in ("q", "k"):
                            for c in range(DCH):
                                nc.tensor.matmul(
                                    pts[name],
                                    w_sb[name][:, c, 0:128],
                                    xht[:, c, :],
                                    start=(c == 0),
                                    stop=False,
                                )
                        for name in ("q", "k"):
                            for c in range(DCH):
                                nc.tensor.matmul(
                                    pts[name],
                                    w_sb[name][:, c, 128:256],
                                    xht[:, c, :],
                                    start=False,
                                    stop=False,
                                )
                            if name == "q":
                                setup_consts()
                        v_proj_and_transpose()
                        for name in ("q", "k"):
                            for c in range(DCH):
                                nc.tensor.matmul(
                                    pts[name],
                                    w_sb[name + "b"][:, c, :],
                                    xlt[:, c, :],
                                    start=False,
                                    stop=(c == DCH - 1),
                                )
                            qk_copies(name, pts[name])
                    else:
                        half = DCH // 2
                        for cs in (slice(0, half), slice(half, DCH)):
                            nc.sync.dma_start(out=xht[:, cs, :],
                                              in_=xt_r[:, cs, sl])
                            nc.sync.dma_start(out=xlt[:, cs, :],
                                              in_=xlb_r[:, cs, sl])
                        if nchunk == 2:
                            nc.sync.dma_start(out=wo_sb, in_=wo[:, :])
                        # on the last chunk the lo-residual copies and the
                        # v work are deferred behind the max-pass tail, so
                        # chunk 0's -c_q row is staged as early as
                        # possible before attention starts (they are only
                        # read late in attention(0))
                        last = pos == NQP - 1
                        deferred = [] if last else None
                        for name in ("q", "k"):
                            pt = ps.tile([128, QCP], F32, tag="ps")
                            i = 0
                            # exact split: xh@wh + xl@wh_bf16 + xh@wl
                            # (weight cols 0:128 = hi both heads,
                            # 128:256 = lo; xh = raw x as f32r)
                            for c in range(DCH):
                                for wop, xt_ in (
                                        (w_sb[name][:, c, 0:128], xht),
                                        (w_sb[name + "b"][:, c, :], xlt),
                                        (w_sb[name][:, c, 128:256], xht)):
                                    nc.tensor.matmul(
                                        pt,
                                        wop,
                                        xt_[:, c, :],
                                        start=(i == 0),
                                        stop=(i == 3 * DCH - 1),
                                    )
                                    i += 1
                            qk_copies(name, pt, deferred)
                        if last:
                            for j in sorted(mp_left):
                                for h in range(2):
                                    mp_pair(0, j, h)
                            mp_left.clear()
                            mp_finish(0)
                            for f in deferred:
                                f()
                        v_proj_and_transpose()
                    # chunk 0 of the max pass rides along with phase 1:
                    # pair j needs exactly k chunk j plus q chunks 0-1
                    done.add(nchunk)
                    if pos >= 1 and pos < NQP - 1 and {0, 1} <= done:
                        for j in sorted(mp_left & done):
                            mp_left.discard(j)
                            for h in range(2):
                                mp_pair(0, j, h)

        def attention_chunk(qc, fillers, seq_heads=False):
            """One q-chunk's attention.  `fillers` is a list of closures
            (next chunk's max-pass pairs, previous chunk's o_proj blocks)
            consumed one per m-block iteration, spreading their PE/DVE/
            Pool load evenly through the chunk."""
            qsl = slice(qc * QC, (qc + 1) * QC)
            ctx_ps = [ps.tile([65, QC], F32, tag="ps", name=f"ctx_ps{h}")
                      for h in range(2)]
            heads_order = ([(mb, h) for mb in range(NMB) for h in range(2)]
                           if not seq_heads else
                           [(mb, h) for h in range(2) for mb in range(NMB)])
            fillers = list(fillers)

            def emit_m1_tail(sp, mb, h):
                # the only matmul that reads row 64 (the -max row); lagging
                # it one m-block behind the cross matmul hides the max
                # staging latency at chunk entry
                nc.tensor.matmul(
                    sp, kT_ext[h][:, mb * 128:(mb + 1) * 128],
                    qT_ext[h][:, qsl],
                    start=False, stop=True,
                )
                et = ex_pool.tile([128, QC], F32R, tag="et", name="et")
                nc.scalar.activation(
                    out=et, in_=sp,
                    func=mybir.ActivationFunctionType.Exp, scale=0.125,
                )
                nc.tensor.matmul(
                    ctx_ps[h], v_ext[h][:, mb, :], et,
                    start=(mb == 0), stop=(mb == NMB - 1),
                )

            lagged = []
            for it, (mb, h) in enumerate(heads_order):
                if it >= 2 and fillers:
                    # drain the filler list ~4 iterations before the chunk
                    # ends so the next chunk's -c_q staging (the last
                    # filler) has settled before its first ext matmul
                    left = max(1, len(heads_order) - 2 - it)
                    n = max(1, -(-len(fillers) // left))
                    for _ in range(n):
                        if fillers:
                            f = fillers.pop(0)
                            if f is not None:
                                f()
                msl = slice(mb * 128, (mb + 1) * 128)
                sp = ps.tile([128, QC], F32, tag="ps", name=f"sp{h}")
                # stacked cross terms first (no row-64 dependency):
                # one K=128 matmul = kl@qh + kh@ql
                nc.tensor.matmul(
                    sp, kx[h][:, msl], qx[h][:, qsl],
                    start=True, stop=False,
                )
                lagged.append((sp, mb, h))
                # lag 2 keeps an extra score tile in flight so the exp
                # latency never backs into the PE; chunk 0 lags deeper
                # still since its -c_q row lands only at the very end of
                # phase 1
                lag = 3
                if len(lagged) > lag:
                    emit_m1_tail(*lagged.pop(0))
                if seq_heads and mb == NMB - 1:
                    while lagged:
                        emit_m1_tail(*lagged.pop(0))
                    if h == 0:
                        # head 1's normalize is emitted by the caller in
                        # column halves, pipelined with its o_proj
                        norm_head(qc, h, ctx_ps)
            while lagged:
                emit_m1_tail(*lagged.pop(0))
            for f in fillers:
                f()
            return ctx_ps

        def norm_head(qc, h, ctx_ps, cl=0, ch=QC):
            q0 = qc * QC
            # normalize: 1/Z broadcast over partitions on Pool
            rz = bc_pool.tile([1, QC], F32, tag="rz")
            nc.vector.reciprocal(out=rz[:, cl:ch], in_=ctx_ps[h][64:65, cl:ch])
            bc_sb = bc_pool.tile([64, QC], F32, tag="bc_sb")
            nc.gpsimd.partition_broadcast(bc_sb[:, cl:ch], rz[:, cl:ch])
            nc.vector.tensor_mul(
                ctxn[h * 64:(h + 1) * 64, q0 + cl:q0 + ch],
                ctx_ps[h][0:64, cl:ch], bc_sb[:, cl:ch]
            )

        _po_eng = [0]

        def oproj_block(qc, nb, dc):
            # one [128, 512] block of this q-chunk's o_proj (both heads
            # fused: K=128), staged through SBUF and DMA'd immediately
            n0 = qc * QC + nb * 128
            po = ps.tile([128, QC], F32, tag="ps", name="po")
            nc.tensor.matmul(
                po,
                ctxn[:, n0:n0 + 128],
                wo_sb[:, dc * QC:(dc + 1) * QC],
                start=True,
                stop=True,
            )
            po_sb = ex_pool.tile([128, QC], BF16, tag="po_sb", bufs=4)
            # staging copies mostly on ACT in chunks that carry max-pass
            # fillers (DVE runs the max chain there); the final chunk has
            # no max-pass, so DVE takes most copies instead
            i = _po_eng[0]
            _po_eng[0] += 1
            dve_share = 3 if qc == NQ - 2 else 0
            if i % 4 < dve_share:
                nc.vector.tensor_copy(po_sb, po)
            else:
                nc.scalar.copy(out=po_sb, in_=po)
            nc.sync.dma_start(
                out=out[n0:n0 + 128, dc * QC:(dc + 1) * QC],
                in_=po_sb)

        def oproj_fillers(qc):
            return [
                (lambda nb=nb, dc=dc: oproj_block(qc, nb, dc))
                for nb in range(QC // 128) for dc in range(D // QC)
            ]

        # pipeline with 1-chunk max-pass lookahead: chunk 0's pairs and
        # finish were hoisted into phase 1; chunk qc+1's pairs + finish
        # ride as early fillers inside attention(qc), chunk qc-1's
        # o_proj blocks as late fillers (their first matmul reads
        # ctxn(qc-1), whose normalize only drains at the start of chunk
        # qc).  The last chunk runs its heads sequentially so head 0's
        # normalize chain overlaps head 1's attention, and its own
        # normalize + o_proj run in column halves to shorten the tail.
        prev = None
        for qc in range(NQ):
            seq = qc == NQ - 1
            mp_f = ([(lambda mb=mb, h=h: mp_block(qc + 1, mb, h))
                     for mb in range(NMB) for h in range(2)]
                    if qc + 1 < NQ else [])
            op_f = oproj_fillers(prev) if prev is not None else []
            # interleave: single-matmul max-pass fillers (each holds just
            # one psum bank until its DVE merge) spread the serial DVE
            # chain; o_proj fillers (ACT-leaning) slot between them, the
            # first held back ~4 iterations until the previous chunk's
            # normalize has drained
            fillers = mp_f[:8] if mp_f else [None] * 8
            rest = mp_f[8:]
            while rest or op_f:
                if op_f:
                    fillers.append(op_f.pop(0))
                for _ in range(2):
                    if rest:
                        fillers.append(rest.pop(0))
            if mp_f:
                fillers.append(lambda: mp_finish(qc + 1))
            ctx_ps = attention_chunk(qc, fillers, seq_heads=seq)
            if not seq:
                for h in range(2):
                    norm_head(qc, h, ctx_ps)
            prev = qc
        # final chunk drain: normalize head 1 in column quarters with the
        # reciprocal / broadcast / multiply stages interleaved (each stage
        # on its engine pipelines across quarters, so the first o_proj
        # block starts after one quarter's chain, not the whole chunk's),
        # then per 128-row block: two matmuls, two staging copies on
        # different engines, ONE [128, 1024] DMA (fewer tail issues)
        q0 = prev * QC
        rz = bc_pool.tile([1, QC], F32, tag="rz")
        bc_sb = bc_pool.tile([64, QC], F32, tag="bc_sb")
        quarters = [slice(i * 128, (i + 1) * 128) for i in range(4)]
        for qs in quarters:
            nc.vector.reciprocal(out=rz[:, qs], in_=ctx_ps[1][64:65, qs])
        for qs in quarters:
            nc.gpsimd.partition_broadcast(bc_sb[:, qs], rz[:, qs])
        for qs in quarters:
            nc.vector.tensor_mul(
                ctxn[64:128, q0 + qs.start:q0 + qs.stop],
                ctx_ps[1][0:64, qs], bc_sb[:, qs])
        for nb in range(QC // 128):
            n0 = prev * QC + nb * 128
            po_nb = ex_pool.tile([128, D], BF16, tag="po_nb", bufs=4,
                                 name="po_nb")
            for dc in range(D // QC):
                po = ps.tile([128, QC], F32, tag="ps", name="po")
                nc.tensor.matmul(
                    po,
                    ctxn[:, n0:n0 + 128],
                    wo_sb[:, dc * QC:(dc + 1) * QC],
                    start=True,
                    stop=True,
                )
                if dc % 2 == 0:
                    nc.vector.tensor_copy(po_nb[:, dc * QC:(dc + 1) * QC], po)
                else:
                    nc.scalar.copy(out=po_nb[:, dc * QC:(dc + 1) * QC],
                                   in_=po)
            dma_eng = (nc.sync, nc.scalar)[nb % 2]
            dma_eng.dma_start(out=out[n0:n0 + 128, :], in_=po_nb)

    nc.compile()
    return nc


def _round11(x):
    # round-to-nearest-even to 11 explicit mantissa bits - exactly the
    # hardware's float32r operand rounding (verified on device)
    u = np.ascontiguousarray(x, dtype=np.float32).view(np.uint32)
    shift = 23 - 11
    add = np.uint32((1 << (shift - 1)) - 1)
    lsb = (u >> np.uint32(shift)) & np.uint32(1)
    mask = np.uint32(~((1 << shift) - 1) & 0xFFFFFFFF)
    return ((u + add + lsb) & mask).view(np.float32)


def _split11(x):
    hi = _round11(x)
    lo = _round11(x.astype(np.float32) - hi)
    return hi, lo


def kernel(x, q_proj, k_proj, v_proj, o_proj):
    import ml_dtypes

    if "nc" not in _CACHE:
        _CACHE["nc"] = build_nc()
    nc = _CACHE["nc"]

    bf16 = ml_dtypes.bfloat16
    xT = np.ascontiguousarray(x.T.astype(np.float32, copy=False))
    # the device uses the raw fp32 bits as the f32r hi operand (PE
    # RNE-11 operand rounding == _round11); ship the lo residual in bf16
    xlbT = np.ascontiguousarray((xT - _round11(xT)).astype(bf16))
    in_maps = []
    for core in range(N_CORES):
        h0 = core * H_PER_CORE

        def wpair(w):
            w2 = np.concatenate([w[h0], w[h0 + 1]], axis=1)  # [D, 128]
            wh, wl = _split11(w2)
            return (np.ascontiguousarray(np.concatenate([wh, wl], axis=1)),
                    np.ascontiguousarray(w2.astype(bf16)))

        wq2, wqb2 = wpair(q_proj)
        wk2, wkb2 = wpair(k_proj)
        m = {
            "xt": xT,
            "xlb": xlbT,
            "wq": wq2,
            "wqb": wqb2,
            "wk": wk2,
            "wkb": wkb2,
            "wv": np.ascontiguousarray(
                np.concatenate([v_proj[h0], v_proj[h0 + 1]], axis=1)),
            "wo": np.ascontiguousarray(o_proj[h0 * 64:(h0 + 2) * 64, :]),
        }
        in_maps.append(m)

    try:
        res = run_bass_kernel_spmd(nc, in_maps, core_ids=list(range(N_CORES)))
    except Exception:
        # one retry: a fresh NRT session recovers transient device faults
        res = run_bass_kernel_spmd(nc, in_maps, core_ids=list(range(N_CORES)))
    _CACHE["last_results"] = res
    acc = np.zeros((N, D), dtype=np.float64)
    for core in range(N_CORES):
        acc += res.results[core]["out"].astype(np.float64)
    return acc.astype(np.float32)


if __name__ == "__main__":
    rng = np.random.default_rng(0)
    ins = {
        "x": rng.standard_normal((N, D), dtype=np.float32),
        "q_proj": rng.standard_normal((H, D, E), dtype=np.float32),
        "k_proj": rng.standard_normal((H, D, E), dtype=np.float32),
        "v_proj": rng.standard_normal((H, D, E), dtype=np.float32),
        "o_proj": rng.standard_normal((D, D), dtype=np.float32),
    }
    out = kernel(**ins)
    print("out", out.shape, out.dtype, np.abs(out).max())
